# revision 1
# baseline (speedup 1.0000x reference)
"""Chamfer distance (CDLoss) Trainium2 kernel.

Problem: prediction [4, 8192, 3], ground_truth [4, 8192, 3] (fp32).
For each batch: d2[n,m] = max(||p_n||^2 + ||g_m||^2 - 2 p.g, 0);
out[b] = sum_n min_m d2 / N + sum_m min_n d2 / M.

Strategy (8 NeuronCores): core c handles (batch = c//2, row-half = c%2),
i.e. a 4096 x 8192 slab of the distance matrix.

Device kernel per core (32 row blocks x 16 column tiles of [128, 512]):
  - Augmented-coordinate trick: ap[5, 4096] = [px, py, pz, ||p||^2, 1],
    ag[5, 8192] = [-2gx, -2gy, -2gz, 1, ||g||^2] so a single K=5 fp32
    matmul emits a [128, 512] tile of squared distances into one PSUM
    bank (PE time ~N cycles regardless of K).
  - VectorE tensor_reduce(min) per tile: exact fp32 row-min partial per
    (rowblock, coltile) into rowparts[128, 32*16].
  - ScalarE copy: PSUM -> SBUF cast to bf16 (the only other PSUM exit).
  - VectorE tensor_tensor(min) in bf16 (2x perf mode): running
    column-min buffer [128, 8192]. bf16 min is exact-monotone
    (min of rounded = rounded min), and the final sum of 8192 values
    concentrates the rounding to ~1e-5 relative.
Host: final tiny reductions (min over 128 partitions / 16 col tiles,
relu clamp, sums) in numpy. min-then-clamp == clamp-then-min, so the
relu of the reference moves to the host gather.
"""

import numpy as np

_B = 4
_N = 8192  # points per cloud
_HALF = _N // 2  # rows per core
_RB = _HALF // 128  # 32 row blocks
_GW = 512  # column group width (one PSUM bank)
_G = _N // _GW  # 16 column groups
_NCORES = 8

_CACHED_NC = None
_RUNNERS = {}


def _build_nc(repeat=1, variant="v1", gw=None, sbufs=6):
    import concourse.bacc as bacc
    import concourse.tile as tile
    from concourse import mybir

    f32 = mybir.dt.float32
    bf16 = mybir.dt.bfloat16

    gw = gw or _GW
    n_g = _N // gw
    n_mm = gw // 512  # matmuls (N<=512 fp32) per column group

    nc = bacc.Bacc("TRN2", target_bir_lowering=False, debug=False)

    ap_d = nc.dram_tensor("ap", [5, _HALF], f32, kind="ExternalInput")
    ag_d = nc.dram_tensor("ag", [5, _N], f32, kind="ExternalInput")
    rowparts_d = nc.dram_tensor(
        "rowparts", [128, _RB * n_g], f32, kind="ExternalOutput"
    )
    colmin_d = nc.dram_tensor("colmin", [128, _N], bf16, kind="ExternalOutput")

    BIG = 1.0e38

    with tile.TileContext(nc) as tc:
        with (
            tc.tile_pool(name="singles", bufs=1) as singles,
            tc.tile_pool(name="spool", bufs=sbufs) as spool,
            tc.tile_pool(name="rpool", bufs=3) as rpool,
            tc.tile_pool(name="psum", bufs=8 // n_mm, space="PSUM") as pp,
        ):
            ap_s = singles.tile([5, _HALF], f32)
            nc.sync.dma_start(out=ap_s[:], in_=ap_d[:])
            ag_s = singles.tile([5, _N], f32)
            nc.sync.dma_start(out=ag_s[:], in_=ag_d[:])

            colmin_s = singles.tile([128, _N], bf16)
            nc.vector.memset(colmin_s[:], BIG)
            rowparts_s = singles.tile([128, _RB * n_g], f32)
            if variant == "v3":
                # v3 only writes one rowparts column per row block; fill
                # the rest with BIG so the host-side min ignores them.
                nc.vector.memset(rowparts_s[:], BIG)

            def _body():
                for rb in range(_RB):
                    lhsT = ap_s[:, rb * 128 : (rb + 1) * 128]
                    if variant == "v3":
                        rowbuf = rpool.tile([128, gw], bf16, tag="rowbuf")
                        nc.vector.memset(rowbuf[:], BIG)
                    for g in range(n_g):
                        t = pp.tile([128, gw], f32, tag="t")
                        for j in range(n_mm):
                            c0 = g * gw + j * 512
                            nc.tensor.matmul(
                                t[:, j * 512 : (j + 1) * 512],
                                lhsT,
                                ag_s[:, c0 : c0 + 512],
                                start=True,
                                stop=True,
                            )
                        idx = rb * n_g + g
                        if variant == "v1":
                            # exact fp32 row minima (DVE, PSUM src, 1x)
                            nc.vector.tensor_reduce(
                                rowparts_s[:, idx : idx + 1],
                                t[:],
                                axis=mybir.AxisListType.X,
                                op=mybir.AluOpType.min,
                            )
                        # PSUM -> SBUF exit on ScalarE, cast to bf16
                        s = spool.tile([128, gw], bf16, tag="s")
                        nc.scalar.copy(s[:], t[:])
                        if variant == "v4":
                            # row minima from the bf16 copy (SBUF src is
                            # cheaper for DVE than fp32 PSUM)
                            nc.vector.tensor_reduce(
                                rowparts_s[:, idx : idx + 1],
                                s[:],
                                axis=mybir.AxisListType.X,
                                op=mybir.AluOpType.min,
                            )
                        if variant == "v3":
                            # row minima via cheap bf16 2x tensor_tensor
                            nc.vector.tensor_tensor(
                                rowbuf[:], rowbuf[:], s[:],
                                op=mybir.AluOpType.min,
                            )
                        # running column minima (bf16, DVE 2x mode)
                        cslice = colmin_s[:, g * gw : (g + 1) * gw]
                        nc.vector.tensor_tensor(
                            cslice, cslice, s[:], op=mybir.AluOpType.min
                        )
                    if variant == "v3":
                        nc.vector.tensor_reduce(
                            rowparts_s[:, rb * n_g : rb * n_g + 1],
                            rowbuf[:],
                            axis=mybir.AxisListType.X,
                            op=mybir.AluOpType.min,
                        )

            if repeat == 1:
                _body()
            else:
                # benchmark mode: body is idempotent (mins), repeat on-device
                with tc.For_i(0, repeat, 1):
                    _body()

            nc.sync.dma_start(out=rowparts_d[:], in_=rowparts_s[:])
            nc.sync.dma_start(out=colmin_d[:], in_=colmin_s[:])

    nc.compile()
    return nc


def _get_nc():
    global _CACHED_NC
    if _CACHED_NC is None:
        _CACHED_NC = _build_nc()
    return _CACHED_NC


def _prep_core_inputs(prediction, ground_truth):
    """Build per-core augmented matrices (host-side, fp32)."""
    in_maps = []
    for c in range(_NCORES):
        b, h = divmod(c, 2)
        p = np.asarray(prediction[b, h * _HALF : (h + 1) * _HALF], dtype=np.float32)
        g = np.asarray(ground_truth[b], dtype=np.float32)
        ap = np.empty((5, _HALF), dtype=np.float32)
        ap[0:3] = p.T
        ap[3] = (p * p).sum(axis=1, dtype=np.float32)
        ap[4] = 1.0
        ag = np.empty((5, _N), dtype=np.float32)
        ag[0:3] = (-2.0 * g).T
        ag[3] = 1.0
        ag[4] = (g * g).sum(axis=1, dtype=np.float32)
        in_maps.append({"ap": ap, "ag": ag})
    return in_maps


def _make_runner(nc, n_cores):
    """Build a cached jitted SPMD executor for `nc` (axon/PJRT path).

    Mirrors concourse.bass2jax.run_bass_via_pjrt but caches the jitted
    callable so repeat calls don't re-trace/re-compile.
    """
    import jax
    import numpy as _np
    from jax.sharding import Mesh, PartitionSpec
    from jax.experimental.shard_map import shard_map
    from concourse import mybir
    from concourse.bass2jax import (
        _bass_exec_p,
        install_neuronx_cc_hook,
        partition_id_tensor,
    )

    install_neuronx_cc_hook()

    partition_name = (
        nc.partition_id_tensor.name if nc.partition_id_tensor else None
    )
    in_names, out_names, out_avals, zero_shapes = [], [], [], []
    for alloc in nc.m.functions[0].allocations:
        if not isinstance(alloc, mybir.MemoryLocationSet):
            continue
        name = alloc.memorylocations[0].name
        if alloc.kind == "ExternalInput":
            if name == partition_name:
                continue
            in_names.append(name)
        elif alloc.kind == "ExternalOutput":
            shape = tuple(alloc.tensor_shape)
            dtype = mybir.dt.np(alloc.dtype)
            out_names.append(name)
            out_avals.append(jax.core.ShapedArray(shape, dtype))
            zero_shapes.append((shape, dtype))
    n_params = len(in_names)
    n_outs = len(out_names)
    all_names = in_names + out_names
    if partition_name is not None:
        all_names = all_names + [partition_name]
    donate = tuple(range(n_params, n_params + n_outs))

    def _body(*args):
        operands = list(args)
        if partition_name is not None:
            operands.append(partition_id_tensor())
        outs = _bass_exec_p.bind(
            *operands,
            out_avals=tuple(out_avals),
            in_names=tuple(all_names),
            out_names=tuple(out_names),
            lowering_input_output_aliases=(),
            sim_require_finite=True,
            sim_require_nnan=True,
            nc=nc,
        )
        return tuple(outs)

    devices = jax.devices()[:n_cores]
    mesh = Mesh(_np.asarray(devices), ("core",))
    sharded = jax.jit(
        shard_map(
            _body,
            mesh=mesh,
            in_specs=(PartitionSpec("core"),) * (n_params + n_outs),
            out_specs=(PartitionSpec("core"),) * n_outs,
            check_rep=False,
        ),
        donate_argnums=donate,
        keep_unused=True,
    )

    def run(in_maps):
        concat_in = [
            _np.concatenate([m[name] for m in in_maps], axis=0)
            for name in in_names
        ]
        concat_zeros = [
            _np.zeros((n_cores * s[0], *s[1:]), d) for (s, d) in zero_shapes
        ]
        out_arrs = sharded(*concat_in, *concat_zeros)
        return [
            {
                name: _np.asarray(out_arrs[i]).reshape(
                    n_cores, *out_avals[i].shape
                )[c]
                for i, name in enumerate(out_names)
            }
            for c in range(n_cores)
        ]

    return run


def _get_runner(nc, n_cores=_NCORES):
    key = id(nc)
    if key not in _RUNNERS:
        _RUNNERS[key] = _make_runner(nc, n_cores)
    return _RUNNERS[key]


def kernel(prediction, ground_truth):
    prediction = np.asarray(prediction, dtype=np.float32)
    ground_truth = np.asarray(ground_truth, dtype=np.float32)

    nc = _get_nc()
    in_maps = _prep_core_inputs(prediction, ground_truth)
    results = _get_runner(nc)(in_maps)

    out = np.zeros(_B, dtype=np.float32)
    for b in range(_B):
        dx = 0.0
        cms = []
        for h in range(2):
            r = results[2 * b + h]
            # rowparts[p, rb*G + g] = min over group g of row rb*128+p
            rp = r["rowparts"].reshape(128, _RB, _G).min(axis=2)  # [128, RB]
            dx += np.maximum(rp, 0.0).sum(dtype=np.float64)
            # colmin[p, j] = min over this core's row-blocks (partition p)
            cms.append(r["colmin"].astype(np.float32).min(axis=0))  # [N]
        cm = np.minimum(cms[0], cms[1])
        dy = np.maximum(cm, 0.0).sum(dtype=np.float64)
        out[b] = dx / _N + dy / _N
    return out



# revision 4
# speedup vs baseline: 1.9337x; 1.9337x over previous
"""Chamfer distance (CDLoss) Trainium2 kernel — v5.

Problem: prediction [4, 8192, 3], ground_truth [4, 8192, 3] (fp32).
For each batch: d2[n,m] = max(||p_n||^2 + ||g_m||^2 - 2 p.g, 0);
out[b] = sum_n min_m d2 / N + sum_m min_n d2 / M.

Sharding (8 NeuronCores): core c handles (batch = c//2, row-half = c%2),
i.e. a 4096 x 8192 slab of the distance matrix.

Device kernel per core — loop over 32 row blocks x 4 column groups
(PSUM tiles [128, 2048], double buffered = 8 banks):
  - Augmented-coordinate trick: ap[5, 4096] = [px, py, pz, ||p||^2, 1],
    ag[5, 8192] = [-2gx, -2gy, -2gz, 1, ||g||^2]; K=5 matmuls in
    *float32r* (1 cycle/row at >=256 moving cols vs 4 for plain fp32)
    emit [128, 512] d2 sub-tiles into PSUM.
  - ScalarE: the only PSUM exit — copy [128, 2048] PSUM fp32 -> SBUF
    fp16 (fp16 keeps d2 to ~5e-4 rel, well within tolerance).
  - VectorE (all fp16 tensor_tensor => 2x perf mode, 0.5 cyc/elem):
      col path: running colmin[128, 8192] = min(colmin, s) per group;
      row path: fold s [128,2048] -> [128,1024] (tt of the two halves)
      then combine into a per-rowblock rowbuf[128, 1024]; one 1x
      tensor_reduce(min) -> rowparts[:, rb] per row block.
  - Optional offloads (knobs): gpsimd (Pool engine) takes one colmin
    tensor_tensor per row block and/or the tail reduce; a few exits can
    move to VectorE tensor_copy to shave ScalarE time.
Host: tiny finals — min over 128 partitions / relu / sums in numpy
(min-then-clamp == clamp-then-min).
"""

import numpy as np

_B = 4
_N = 8192  # points per cloud
_HALF = _N // 2  # rows per core
_RB = _HALF // 128  # 32 row blocks
_GW = 2048  # column group width (4 PSUM banks)
_G = _N // _GW  # 4 column groups
_NCORES = 8

_CACHED_NC = None
_RUNNERS = {}


def _build_nc(repeat=1, mm="f32r", pool_col=0, pool_reduce=False,
              dve_exit_gis=()):
    """Build the per-core Bass program.

    mm: "f32r" | "f32" — matmul input dtype view.
    pool_col: how many of the 4 colmin tensor_tensors per row block run
      on the gpsimd (Pool) engine instead of VectorE.
    pool_reduce: run the per-rowblock tail tensor_reduce on gpsimd.
    dve_exit_gis: set of gi indices whose PSUM->SBUF exit runs on
      VectorE (tensor_copy) instead of ScalarE.
    """
    import concourse.bacc as bacc
    import concourse.tile as tile
    from concourse import mybir

    f32 = mybir.dt.float32
    f32r = mybir.dt.float32r
    fp16 = mybir.dt.float16

    nc = bacc.Bacc("TRN2", target_bir_lowering=False, debug=False)

    in_dt = f32r if mm == "f32r" else f32
    ap_d = nc.dram_tensor("ap", [5, _HALF], in_dt, kind="ExternalInput")
    ag_d = nc.dram_tensor("ag", [5, _N], in_dt, kind="ExternalInput")
    rowparts_d = nc.dram_tensor(
        "rowparts", [128, _RB], f32, kind="ExternalOutput"
    )
    colmin_d = nc.dram_tensor("colmin", [128, _N], fp16, kind="ExternalOutput")

    BIG = 60000.0
    MIN = mybir.AluOpType.min
    AXX = mybir.AxisListType.X

    with tile.TileContext(nc) as tc:
        with (
            tc.tile_pool(name="singles", bufs=1) as singles,
            tc.tile_pool(name="spool", bufs=3) as spool,
            tc.tile_pool(name="rpool", bufs=2) as rpool,
            tc.tile_pool(name="tpool", bufs=2) as tpool,
            tc.tile_pool(name="psum", bufs=2, space="PSUM") as pp,
        ):
            ap_s = singles.tile([5, _HALF], in_dt)
            nc.sync.dma_start(out=ap_s[:], in_=ap_d[:])
            ag_s = singles.tile([5, _N], in_dt)
            nc.sync.dma_start(out=ag_s[:], in_=ag_d[:])

            colmin_s = singles.tile([128, _N], fp16)
            nc.vector.memset(colmin_s[:], BIG)
            rowparts_s = singles.tile([128, _RB], f32)

            ap_mm = ap_s[:]
            ag_mm = ag_s[:]

            def _body():
                for rb in range(_RB):
                    lhsT = ap_mm[:, rb * 128 : (rb + 1) * 128]
                    rbuf = rpool.tile([128, _GW // 2], fp16, tag="rbuf")
                    for gi in range(_G):
                        t = pp.tile([128, _GW], f32, tag="t")
                        for j in range(_GW // 512):
                            c0 = gi * _GW + j * 512
                            nc.tensor.matmul(
                                t[:, j * 512 : (j + 1) * 512],
                                lhsT,
                                ag_mm[:, c0 : c0 + 512],
                                start=True,
                                stop=True,
                            )
                        # PSUM exit: fp32 -> fp16
                        s = spool.tile([128, _GW], fp16, tag="s")
                        if gi in dve_exit_gis:
                            nc.vector.tensor_copy(s[:], t[:])
                        else:
                            nc.scalar.copy(s[:], t[:])
                        # row path: fold halves, then combine
                        h = _GW // 2
                        if gi == 0:
                            nc.vector.tensor_tensor(
                                rbuf[:], s[:, :h], s[:, h:], op=MIN
                            )
                        else:
                            tmp = tpool.tile([128, h], fp16, tag="tmp")
                            nc.vector.tensor_tensor(
                                tmp[:], s[:, :h], s[:, h:], op=MIN
                            )
                            nc.vector.tensor_tensor(
                                rbuf[:], rbuf[:], tmp[:], op=MIN
                            )
                        # col path: running colmin
                        cslice = colmin_s[:, gi * _GW : (gi + 1) * _GW]
                        eng = nc.gpsimd if gi < pool_col else nc.vector
                        eng.tensor_tensor(cslice, cslice, s[:], op=MIN)
                    red_eng = nc.gpsimd if pool_reduce else nc.vector
                    red_eng.tensor_reduce(
                        rowparts_s[:, rb : rb + 1],
                        rbuf[:],
                        axis=AXX,
                        op=MIN,
                    )

            if repeat == 1:
                _body()
            else:
                # benchmark mode: body is idempotent (mins), repeat on-device
                with tc.For_i(0, repeat, 1):
                    _body()

            nc.sync.dma_start(out=rowparts_d[:], in_=rowparts_s[:])
            nc.sync.dma_start(out=colmin_d[:], in_=colmin_s[:])

    nc.compile()
    return nc


def _get_nc():
    global _CACHED_NC
    if _CACHED_NC is None:
        _CACHED_NC = _build_nc()
    return _CACHED_NC


def _prep_core_inputs(prediction, ground_truth):
    """Build per-core augmented matrices (host-side, fp32)."""
    in_maps = []
    for c in range(_NCORES):
        b, h = divmod(c, 2)
        p = np.asarray(prediction[b, h * _HALF : (h + 1) * _HALF], dtype=np.float32)
        g = np.asarray(ground_truth[b], dtype=np.float32)
        ap = np.empty((5, _HALF), dtype=np.float32)
        ap[0:3] = p.T
        ap[3] = (p * p).sum(axis=1, dtype=np.float32)
        ap[4] = 1.0
        ag = np.empty((5, _N), dtype=np.float32)
        ag[0:3] = (-2.0 * g).T
        ag[3] = 1.0
        ag[4] = (g * g).sum(axis=1, dtype=np.float32)
        in_maps.append({"ap": ap, "ag": ag})
    return in_maps


def _make_runner(nc, n_cores):
    """Build a cached jitted SPMD executor for `nc` (axon/PJRT path).

    Mirrors concourse.bass2jax.run_bass_via_pjrt but caches the jitted
    callable so repeat calls don't re-trace/re-compile.
    """
    import jax
    import numpy as _np
    from jax.sharding import Mesh, PartitionSpec
    from jax.experimental.shard_map import shard_map
    from concourse import mybir
    from concourse.bass2jax import (
        _bass_exec_p,
        install_neuronx_cc_hook,
        partition_id_tensor,
    )

    install_neuronx_cc_hook()

    partition_name = (
        nc.partition_id_tensor.name if nc.partition_id_tensor else None
    )
    in_names, out_names, out_avals, zero_shapes = [], [], [], []
    for alloc in nc.m.functions[0].allocations:
        if not isinstance(alloc, mybir.MemoryLocationSet):
            continue
        name = alloc.memorylocations[0].name
        if alloc.kind == "ExternalInput":
            if name == partition_name:
                continue
            in_names.append(name)
        elif alloc.kind == "ExternalOutput":
            shape = tuple(alloc.tensor_shape)
            dtype = mybir.dt.np(alloc.dtype)
            out_names.append(name)
            out_avals.append(jax.core.ShapedArray(shape, dtype))
            zero_shapes.append((shape, dtype))
    n_params = len(in_names)
    n_outs = len(out_names)
    all_names = in_names + out_names
    if partition_name is not None:
        all_names = all_names + [partition_name]
    donate = tuple(range(n_params, n_params + n_outs))

    def _body(*args):
        operands = list(args)
        if partition_name is not None:
            operands.append(partition_id_tensor())
        outs = _bass_exec_p.bind(
            *operands,
            out_avals=tuple(out_avals),
            in_names=tuple(all_names),
            out_names=tuple(out_names),
            lowering_input_output_aliases=(),
            sim_require_finite=True,
            sim_require_nnan=True,
            nc=nc,
        )
        return tuple(outs)

    devices = jax.devices()[:n_cores]
    mesh = Mesh(_np.asarray(devices), ("core",))
    sharded = jax.jit(
        shard_map(
            _body,
            mesh=mesh,
            in_specs=(PartitionSpec("core"),) * (n_params + n_outs),
            out_specs=(PartitionSpec("core"),) * n_outs,
            check_rep=False,
        ),
        donate_argnums=donate,
        keep_unused=True,
    )

    def run(in_maps):
        concat_in = [
            _np.concatenate([m[name] for m in in_maps], axis=0)
            for name in in_names
        ]
        concat_zeros = [
            _np.zeros((n_cores * s[0], *s[1:]), d) for (s, d) in zero_shapes
        ]
        out_arrs = sharded(*concat_in, *concat_zeros)
        return [
            {
                name: _np.asarray(out_arrs[i]).reshape(
                    n_cores, *out_avals[i].shape
                )[c]
                for i, name in enumerate(out_names)
            }
            for c in range(n_cores)
        ]

    return run


def _get_runner(nc, n_cores=_NCORES):
    key = id(nc)
    if key not in _RUNNERS:
        _RUNNERS[key] = _make_runner(nc, n_cores)
    return _RUNNERS[key]


def kernel(prediction, ground_truth):
    prediction = np.asarray(prediction, dtype=np.float32)
    ground_truth = np.asarray(ground_truth, dtype=np.float32)

    nc = _get_nc()
    in_maps = _prep_core_inputs(prediction, ground_truth)
    results = _get_runner(nc)(in_maps)

    out = np.zeros(_B, dtype=np.float32)
    for b in range(_B):
        dx = 0.0
        cms = []
        for h in range(2):
            r = results[2 * b + h]
            # rowparts[p, rb] = min over row rb*128+p of this core's slab
            dx += np.maximum(r["rowparts"], 0.0).sum(dtype=np.float64)
            # colmin[p, j] = min over this core's row-blocks at partition p
            cms.append(r["colmin"].astype(np.float32).min(axis=0))  # [N]
        cm = np.minimum(cms[0], cms[1])
        dy = np.maximum(cm, 0.0).sum(dtype=np.float64)
        out[b] = dx / _N + dy / _N
    return out


# revision 12
# speedup vs baseline: 2.3640x; 1.2225x over previous
"""Chamfer distance (CDLoss) Trainium2 kernel — v5.

Problem: prediction [4, 8192, 3], ground_truth [4, 8192, 3] (fp32).
For each batch: d2[n,m] = max(||p_n||^2 + ||g_m||^2 - 2 p.g, 0);
out[b] = sum_n min_m d2 / N + sum_m min_n d2 / M.

Sharding (8 NeuronCores): core c handles (batch = c//2, row-half = c%2),
i.e. a 4096 x 8192 slab of the distance matrix.

Device kernel per core — loop over 32 row blocks x 4 column groups
(PSUM tiles [128, 2048], double buffered = 8 banks):
  - Augmented-coordinate trick: ap[5, 4096] = [px, py, pz, ||p||^2, 1],
    ag[5, 8192] = [-2gx, -2gy, -2gz, 1, ||g||^2]; K=5 matmuls in
    *float32r* (1 cycle/row at >=256 moving cols vs 4 for plain fp32)
    emit [128, 512] d2 sub-tiles into PSUM.
  - ScalarE: the only PSUM exit — copy [128, 2048] PSUM fp32 -> SBUF
    fp16 (fp16 keeps d2 to ~5e-4 rel, well within tolerance).
  - VectorE (all fp16 tensor_tensor => 2x perf mode, 0.5 cyc/elem):
      col path: running colmin[128, 8192] = min(colmin, s) per group;
      row path: fold s [128,2048] -> [128,1024] (tt of the two halves)
      then combine into a per-rowblock rowbuf[128, 1024]; one 1x
      tensor_reduce(min) -> rowparts[:, rb] per row block.
  - Optional offloads (knobs): gpsimd (Pool engine) takes one colmin
    tensor_tensor per row block and/or the tail reduce; a few exits can
    move to VectorE tensor_copy to shave ScalarE time.
Host: tiny finals — min over 128 partitions / relu / sums in numpy
(min-then-clamp == clamp-then-min).
"""

import numpy as np

_B = 4
_N = 8192  # points per cloud
_HALF = _N // 2  # rows per core
_RB = _HALF // 128  # 32 row blocks
_GW = 2048  # column group width (4 PSUM banks)
_G = _N // _GW  # 4 column groups
_NCORES = 8

_CACHED_NC = None
_RUNNERS = {}


def _build_nc(repeat=1, mm="f32r", pool_col=0, pool_reduce=False,
              dve_exit_gis=(), ablate="", variant="v5"):
    """Build the per-core Bass program.

    mm: "f32r" | "f32" — matmul input dtype view.
    pool_col: how many of the 4 colmin tensor_tensors per row block run
      on the gpsimd (Pool) engine instead of VectorE.
    pool_reduce: run the per-rowblock tail tensor_reduce on gpsimd.
    dve_exit_gis: set of gi indices whose PSUM->SBUF exit runs on
      VectorE (tensor_copy) instead of ScalarE.
    """
    import concourse.bacc as bacc
    import concourse.tile as tile
    from concourse import mybir

    f32 = mybir.dt.float32
    f32r = mybir.dt.float32r
    fp16 = mybir.dt.float16

    nc = bacc.Bacc("TRN2", target_bir_lowering=False, debug=False)

    in_dt = f32r if mm == "f32r" else f32
    ap_d = nc.dram_tensor("ap", [5, _HALF], in_dt, kind="ExternalInput")
    ag_d = nc.dram_tensor("ag", [5, _N], in_dt, kind="ExternalInput")
    rowparts_d = nc.dram_tensor(
        "rowparts", [128, _RB], f32, kind="ExternalOutput"
    )
    colmin_d = nc.dram_tensor("colmin", [128, _N], fp16, kind="ExternalOutput")

    BIG = 60000.0
    MIN = mybir.AluOpType.min
    AXX = mybir.AxisListType.X

    with tile.TileContext(nc) as tc:
        with (
            tc.tile_pool(name="singles", bufs=1) as singles,
            tc.tile_pool(name="spool", bufs=3) as spool,
            tc.tile_pool(name="rpool", bufs=2) as rpool,
            tc.tile_pool(name="tpool", bufs=2) as tpool,
            tc.tile_pool(name="psum", bufs=2, space="PSUM") as pp,
        ):
            ap_s = singles.tile([5, _HALF], in_dt)
            nc.sync.dma_start(out=ap_s[:], in_=ap_d[:])
            ag_s = singles.tile([5, _N], in_dt)
            nc.sync.dma_start(out=ag_s[:], in_=ag_d[:])

            colmin_s = singles.tile([128, _N], fp16)
            nc.vector.memset(colmin_s[:], BIG)
            rowparts_s = singles.tile([128, _RB], f32)
            if ablate:
                nc.vector.memset(rowparts_s[:], 0.0)

            ap_mm = ap_s[:]
            ag_mm = ag_s[:]

            def _body_v7():
                # s-supertile per row block; fold-tree row path; fused
                # tensor_tensor_reduce tail.
                for rb in range(_RB):
                    lhsT = ap_mm[:, rb * 128 : (rb + 1) * 128]
                    s = spool.tile([128, _N], fp16, tag="s")
                    for gi in range(_G):
                        t = pp.tile([128, _GW], f32, tag="t")
                        for j in range(_GW // 512):
                            c0 = gi * _GW + j * 512
                            nc.tensor.matmul(
                                t[:, j * 512 : (j + 1) * 512],
                                lhsT,
                                ag_mm[:, c0 : c0 + 512],
                                start=True,
                                stop=True,
                            )
                        if gi in dve_exit_gis:
                            nc.vector.tensor_copy(
                                s[:, gi * _GW : (gi + 1) * _GW], t[:]
                            )
                        else:
                            nc.scalar.copy(
                                s[:, gi * _GW : (gi + 1) * _GW], t[:]
                            )
                    # col path: one running-min over the whole row block
                    nc.vector.tensor_tensor(
                        colmin_s[:], colmin_s[:], s[:], op=MIN
                    )
                    # row path: fold tree 8192 -> 4096 -> 2048, then fused
                    # tt+reduce 2048 -> rowparts[:, rb]
                    f1 = rpool.tile([128, _N // 2], fp16, tag="f1")
                    nc.vector.tensor_tensor(
                        f1[:], s[:, : _N // 2], s[:, _N // 2 :], op=MIN
                    )
                    f2 = tpool.tile([128, _N // 4], fp16, tag="f2")
                    nc.vector.tensor_tensor(
                        f2[:], f1[:, : _N // 4], f1[:, _N // 4 :], op=MIN
                    )
                    f3 = tpool.tile([128, _N // 8], fp16, tag="f3")
                    nc.vector.tensor_tensor_reduce(
                        out=f3[:],
                        in0=f2[:, : _N // 8],
                        in1=f2[:, _N // 8 :],
                        scale=1.0,
                        scalar=BIG,
                        op0=MIN,
                        op1=MIN,
                        accum_out=rowparts_s[:, rb : rb + 1],
                    )

            def _body():
                for rb in range(_RB):
                    lhsT = ap_mm[:, rb * 128 : (rb + 1) * 128]
                    rbuf = None
                    for gi in range(_G):
                        t = pp.tile([128, _GW], f32, tag="t")
                        for j in range(_GW // 512):
                            c0 = gi * _GW + j * 512
                            nc.tensor.matmul(
                                t[:, j * 512 : (j + 1) * 512],
                                lhsT,
                                ag_mm[:, c0 : c0 + 512],
                                start=True,
                                stop=True,
                            )
                        if ablate == "pe":
                            continue
                        # PSUM exit: fp32 -> fp16
                        s = spool.tile([128, _GW], fp16, tag="s")
                        if gi in dve_exit_gis:
                            nc.vector.tensor_copy(s[:], t[:])
                        else:
                            nc.scalar.copy(s[:], t[:])
                        if ablate == "peact":
                            continue
                        # row path: fold halves, then combine
                        h = _GW // 2
                        if ablate != "pecol":
                            if gi == 0:
                                rbuf = rpool.tile(
                                    [128, _GW // 2], fp16, tag="rbuf"
                                )
                                nc.vector.tensor_tensor(
                                    rbuf[:], s[:, :h], s[:, h:], op=MIN
                                )
                            else:
                                tmp = tpool.tile([128, h], fp16, tag="tmp")
                                nc.vector.tensor_tensor(
                                    tmp[:], s[:, :h], s[:, h:], op=MIN
                                )
                                nc.vector.tensor_tensor(
                                    rbuf[:], rbuf[:], tmp[:], op=MIN
                                )
                        # col path: running colmin
                        if ablate != "perow":
                            cslice = colmin_s[:, gi * _GW : (gi + 1) * _GW]
                            eng = nc.gpsimd if gi < pool_col else nc.vector
                            eng.tensor_tensor(cslice, cslice, s[:], op=MIN)
                    if ablate in ("pe", "peact", "pecol"):
                        continue
                    red_eng = nc.gpsimd if pool_reduce else nc.vector
                    red_eng.tensor_reduce(
                        rowparts_s[:, rb : rb + 1],
                        rbuf[:],
                        axis=AXX,
                        op=MIN,
                    )

            body = _body_v7 if variant == "v7" else _body
            if repeat == 1:
                body()
            else:
                # benchmark mode: body is idempotent (mins), repeat on-device
                with tc.For_i(0, repeat, 1):
                    body()

            nc.sync.dma_start(out=rowparts_d[:], in_=rowparts_s[:])
            nc.sync.dma_start(out=colmin_d[:], in_=colmin_s[:])

    nc.compile()
    return nc


def _get_nc():
    global _CACHED_NC
    if _CACHED_NC is None:
        _CACHED_NC = _build_nc()
    return _CACHED_NC


def _prep_core_inputs(prediction, ground_truth):
    """Build per-core augmented matrices (host-side, fp32)."""
    in_maps = []
    for c in range(_NCORES):
        b, h = divmod(c, 2)
        p = np.asarray(prediction[b, h * _HALF : (h + 1) * _HALF], dtype=np.float32)
        g = np.asarray(ground_truth[b], dtype=np.float32)
        ap = np.empty((5, _HALF), dtype=np.float32)
        ap[0:3] = p.T
        ap[3] = (p * p).sum(axis=1, dtype=np.float32)
        ap[4] = 1.0
        ag = np.empty((5, _N), dtype=np.float32)
        ag[0:3] = (-2.0 * g).T
        ag[3] = 1.0
        ag[4] = (g * g).sum(axis=1, dtype=np.float32)
        in_maps.append({"ap": ap, "ag": ag})
    return in_maps


def _make_runner(nc, n_cores):
    """Build a cached jitted SPMD executor for `nc` (axon/PJRT path).

    Mirrors concourse.bass2jax.run_bass_via_pjrt but caches the jitted
    callable so repeat calls don't re-trace/re-compile.
    """
    import jax
    import numpy as _np
    from jax.sharding import Mesh, PartitionSpec
    from jax.experimental.shard_map import shard_map
    from concourse import mybir
    from concourse.bass2jax import (
        _bass_exec_p,
        install_neuronx_cc_hook,
        partition_id_tensor,
    )

    install_neuronx_cc_hook()

    partition_name = (
        nc.partition_id_tensor.name if nc.partition_id_tensor else None
    )
    in_names, out_names, out_avals, zero_shapes = [], [], [], []
    for alloc in nc.m.functions[0].allocations:
        if not isinstance(alloc, mybir.MemoryLocationSet):
            continue
        name = alloc.memorylocations[0].name
        if alloc.kind == "ExternalInput":
            if name == partition_name:
                continue
            in_names.append(name)
        elif alloc.kind == "ExternalOutput":
            shape = tuple(alloc.tensor_shape)
            dtype = mybir.dt.np(alloc.dtype)
            out_names.append(name)
            out_avals.append(jax.core.ShapedArray(shape, dtype))
            zero_shapes.append((shape, dtype))
    n_params = len(in_names)
    n_outs = len(out_names)
    all_names = in_names + out_names
    if partition_name is not None:
        all_names = all_names + [partition_name]
    donate = tuple(range(n_params, n_params + n_outs))

    def _body(*args):
        operands = list(args)
        if partition_name is not None:
            operands.append(partition_id_tensor())
        outs = _bass_exec_p.bind(
            *operands,
            out_avals=tuple(out_avals),
            in_names=tuple(all_names),
            out_names=tuple(out_names),
            lowering_input_output_aliases=(),
            sim_require_finite=True,
            sim_require_nnan=True,
            nc=nc,
        )
        return tuple(outs)

    devices = jax.devices()[:n_cores]
    mesh = Mesh(_np.asarray(devices), ("core",))
    sharded = jax.jit(
        shard_map(
            _body,
            mesh=mesh,
            in_specs=(PartitionSpec("core"),) * (n_params + n_outs),
            out_specs=(PartitionSpec("core"),) * n_outs,
            check_rep=False,
        ),
        donate_argnums=donate,
        keep_unused=True,
    )

    def run(in_maps):
        concat_in = [
            _np.concatenate([m[name] for m in in_maps], axis=0)
            for name in in_names
        ]
        concat_zeros = [
            _np.zeros((n_cores * s[0], *s[1:]), d) for (s, d) in zero_shapes
        ]
        out_arrs = sharded(*concat_in, *concat_zeros)
        return [
            {
                name: _np.asarray(out_arrs[i]).reshape(
                    n_cores, *out_avals[i].shape
                )[c]
                for i, name in enumerate(out_names)
            }
            for c in range(n_cores)
        ]

    return run


def _get_runner(nc, n_cores=_NCORES):
    key = id(nc)
    if key not in _RUNNERS:
        _RUNNERS[key] = _make_runner(nc, n_cores)
    return _RUNNERS[key]


def kernel(prediction, ground_truth):
    prediction = np.asarray(prediction, dtype=np.float32)
    ground_truth = np.asarray(ground_truth, dtype=np.float32)

    nc = _get_nc()
    in_maps = _prep_core_inputs(prediction, ground_truth)
    results = _get_runner(nc)(in_maps)

    out = np.zeros(_B, dtype=np.float32)
    for b in range(_B):
        dx = 0.0
        cms = []
        for h in range(2):
            r = results[2 * b + h]
            # rowparts[p, rb] = min over row rb*128+p of this core's slab
            dx += np.maximum(r["rowparts"], 0.0).sum(dtype=np.float64)
            # colmin[p, j] = min over this core's row-blocks at partition p
            cms.append(r["colmin"].astype(np.float32).min(axis=0))  # [N]
        cm = np.minimum(cms[0], cms[1])
        dy = np.maximum(cm, 0.0).sum(dtype=np.float64)
        out[b] = dx / _N + dy / _N
    return out
